# revision 47
# baseline (speedup 1.0000x reference)
"""ANI-style species-routed MLP (MoE routing) on 8 TRN2 NeuronCores, v3.

Strategy (v3, on top of the v2 baseline):
- Atom-level balanced routing: each species' atoms are dealt round-robin
  across the 8 cores, so every core sees ~ceil(N_s/8) atoms of species s;
  capacity is 16-aligned (880 here).  The per-molecule reduction happens
  on the host, so atoms can be assigned to cores freely.
- Feature-major fp16 matmuls (fp32 PSUM accumulate).  An optional fp8
  path (FP8_L1) runs L1's k0/k1 as one fp8e4 DoubleRow matmul (measured
  2x PE rate at N=512) with k2 kept fp16; it is off by default: it saves
  ~1us on average but costs 1.1e-2 relative error and makes the PE
  outrun the PSUM z-tile rotation (celu chain ~2.7us vs 4-deep pool),
  which re-throttles the HAM clock.
- PE warm-up: 8 dummy matmuls with no DMA dependencies run in the input
  DMA shadow, so the HAM activity window (4096 cycles @1.2GHz) completes
  and the PE runs the entire real stream at 2.4GHz (verified: single
  51us warm stretch in the trace).  The exp/relu ACT table set is also
  preloaded there (otherwise it loads lazily, ~1.3us on first celu).
- CELU via the exact exp/min/max trick:
      u := celu(z+b)+a = max(z + b + a, min(a*e^{10(z+b)}, a))
                       = min(a*e^{10(z+b)}, a) + relu(z+b)
  with V1 (exp ACT; min 4x + stt 1x on DVE) and V4 (exp+relu ACT; min +
  tt-add on DVE) cycled by FORM_PATTERN to balance ACT ~= DVE ~= 44us.
  (scalar_tensor_tensor always runs at 1x -- fusing min+add into one stt
  is a loss; GpSimd tensor ops are ~10-40x slower than DVE on TRN2.)
- Small-M/K matmuls exploit PE 32x32 sub-array concurrency: consecutive
  matmuls on disjoint (row,col) tile_position groups execute with ~3ns
  stagger (measured).  L2 m1 (M=64) packs 2 species on col groups
  {0,64}; L3/L4 m1 (M=32) pack 4 species on col groups {0,32,64,96};
  L3/L4 k1 (K=64/32) also row-tile.  Matmul order within a group is
  k0(all species) then k1(all species) so the concurrent tiles are
  adjacent in the instruction stream with no waits between them.
- Globally interleaved emission: every celu -> consumer edge gets ~3us
  of independent PE work (the celu chain exp->min->stt is ~2.7us after
  the z stop), including splitting L4's k0 matmuls out so they run as
  soon as each species' u3 is ready.
- Energy output ships one [1, cap] row per species (the 32-row blocks
  are replicas); supergroup-0 rows are DMA'd mid-kernel.
"""
import os
import sys

sys.path.insert(0, "/opt/trn_rl_repo")

from contextlib import ExitStack

import numpy as np

import concourse.bass as bass
import concourse.mybir as mybir
import concourse.tile as tile
from concourse import bacc
from concourse.bass_utils import run_bass_kernel_spmd

F32 = mybir.dt.float32
F16 = mybir.dt.float16
F8 = mybir.dt.float8e4
PM = mybir.MatmulPerfMode
AF = mybir.ActivationFunctionType
ALU = mybir.AluOpType

B, A, F = 1024, 48, 384
S = 7
NCORES = 8
ALPHA = 0.1
LN_ALPHA = float(np.log(ALPHA))

PAIRS = ((0, 1), (2, 3), (4, 5), (6,))
QUADS = ((0, 1, 2, 3), (4, 5, 6))
SGROUPS = ((0, 1), (2, 3))  # supergroups: pair indices per L3/L4 block

# --- tuning knobs (cache key includes them) ---
FORM_PATTERN = ("V1", "V4", "V1", "V1", "V1", "V4", "V1",
                "V1", "V4", "V1", "V1", "V1", "V4", "V1")
ZBUFS = 4
TBUFS = 8
U1BUFS = 4
U2BUFS = 7
U2P_BUFS = 4
U3BUFS = 7
U3Q_BUFS = 2
WARMUP_MMS = 6
FP8_L1 = False  # L1 weights+aev in fp8e4 with DoubleRow (k0,k1) + normal k2

_CACHE = {}
LAST_EXEC_NS = None


def _build(cap):
    assert cap % 16 == 0
    halves = [(o, min(512, cap - o)) for o in range(0, cap, 512)]
    nc = bacc.Bacc()

    if FP8_L1:
        # k0,k1 in fp8 (DoubleRow pair); k2 stays fp16 for accuracy --
        # same PE cost, ~sqrt(2/3) of the full-fp8 quantization error
        x8_d = nc.declare_dram_parameter("xt8", [128, S, 2, cap], F8,
                                         isOutput=False)
        w18_d = nc.declare_dram_parameter("w1t8", [128, S, 2, 256], F8,
                                          isOutput=False)
        x16_d = nc.declare_dram_parameter("xt16", [128, S, cap], F16,
                                          isOutput=False)
        w116_d = nc.declare_dram_parameter("w1t16", [128, S, 256], F16,
                                           isOutput=False)
    else:
        xt_d = nc.declare_dram_parameter("xt", [128, S, 3, cap], F16,
                                         isOutput=False)
        w1_d = nc.declare_dram_parameter("w1t", [128, S, 3, 256], F16,
                                         isOutput=False)
    w2_d = nc.declare_dram_parameter("w2t", [128, S, 2, 192], F16, isOutput=False)
    w3_d = nc.declare_dram_parameter("w3t", [128, S, 2, 160], F16, isOutput=False)
    # w4 columns replicated 32x: L4 outputs fill a full 32-partition block
    # (M=32 costs the same as M=1) so the PSUM->SBUF copy reads no stale rows
    w4_d = nc.declare_dram_parameter("w4t", [128, S, 2, 32], F16, isOutput=False)
    # biases [128, S, layer(3), kind(bx,bc,br), m(2)] (m1 slots of L2/L3 unused)
    bb_d = nc.declare_dram_parameter("biases", [128, S, 3, 3, 2], F32,
                                     isOutput=False)
    # packed L2-m1 pair biases [128, npairs, kind]
    bp2_d = nc.declare_dram_parameter("bp2", [128, len(PAIRS), 3], F32,
                                      isOutput=False)
    # packed L3-m1 quad biases [128, nquads, kind]
    bq3_d = nc.declare_dram_parameter("bq3", [128, len(QUADS), 3], F32,
                                      isOutput=False)
    # energy out: one row per species (partition 32*(s%4) of supergroup s//4)
    en_d = nc.declare_dram_parameter("energy", [S, cap], F32, isOutput=True)

    l1_k = [(0, 128), (128, 128), (256, 128)]
    l2_k = [(0, 128), (128, 128)]
    l3_k = [(0, 128), (128, 64)]
    l4_k = [(0, 128), (128, 32)]

    with tile.TileContext(nc) as tc, ExitStack() as ctx:
        wpool = ctx.enter_context(tc.tile_pool(name="weights", bufs=1))
        xpool = ctx.enter_context(tc.tile_pool(name="x", bufs=S))
        u1pool = ctx.enter_context(tc.tile_pool(name="u1", bufs=U1BUFS))
        u2pool = ctx.enter_context(tc.tile_pool(name="u2", bufs=U2BUFS))
        u2ppool = ctx.enter_context(tc.tile_pool(name="u2p", bufs=U2P_BUFS))
        u3pool = ctx.enter_context(tc.tile_pool(name="u3", bufs=U3BUFS))
        u3qpool = ctx.enter_context(tc.tile_pool(name="u3q", bufs=U3Q_BUFS))
        tpool = ctx.enter_context(tc.tile_pool(name="t", bufs=TBUFS))
        zpool = ctx.enter_context(tc.tile_pool(name="z", bufs=ZBUFS, space="PSUM"))
        epool = ctx.enter_context(tc.tile_pool(name="en", bufs=1))

        # --- PE warm-up: dummy matmuls with no DMA dependency keep the HAM
        # activity window busy from kernel start, so the ramp to K=8/8
        # happens during the input-DMA shadow instead of eating into the
        # real matmul stream. ---
        wdummy = wpool.tile([128, 512], F16)
        nc.vector.memset(wdummy[:], 0.0)
        zwarm = zpool.tile([128, 1024], F32, tag="z", name="zwarm")
        for _ in range(WARMUP_MMS):
            nc.tensor.matmul(zwarm[:, 0:512], wdummy[:, 0:128],
                             wdummy[:], start=True, stop=True)
        # preload the exp/relu ACT table set during the DMA shadow (it
        # otherwise loads lazily at the first celu, ~1.3us on the critical
        # path)
        ed = wpool.tile([128, 8], F16)
        nc.scalar.activation(ed[:], zwarm[:, 0:8], AF.Exp, bias=0.0,
                             scale=1.0)

        # --- DMAs: ordered by first-use time.  Species 0's x/w1 chunks are
        # k-interleaved and issued first so the first L1 matmul can start as
        # early as possible; w3/w4/bq3 are only needed by the tails and go
        # last so they don't starve the early matmul stream. ---
        bb = wpool.tile([128, S, 3, 3, 2], F32)
        bp2 = wpool.tile([128, len(PAIRS), 3], F32)
        bq3 = wpool.tile([128, len(QUADS), 3], F32)

        w2 = wpool.tile([128, S, 2, 192], F16)
        w3 = wpool.tile([128, S, 2, 160], F16)
        w4 = wpool.tile([128, S, 2, 32], F16)
        x8_tiles, x16_tiles = {}, {}
        if FP8_L1:
            w18 = wpool.tile([128, S, 2, 256], F8)
            w116 = wpool.tile([128, S, 256], F16)
            for s in range(S):
                x8_tiles[s] = xpool.tile([128, 2, cap], F8, tag="x8",
                                         name=f"x8_{s}")
                x16_tiles[s] = xpool.tile([128, cap], F16, tag="x16",
                                          name=f"x16_{s}")

            def dma_species(s):
                # fp16 k2 first: the first matmul of each species reads it
                nc.sync.dma_start(w116[:, s], w116_d.ap()[:, s])
                nc.sync.dma_start(x16_tiles[s][:], x16_d.ap()[:, s])
                nc.sync.dma_start(w18[:, s], w18_d.ap()[:, s])
                nc.sync.dma_start(x8_tiles[s][:], x8_d.ap()[:, s])
                if s == 0:
                    nc.sync.dma_start(bb[:], bb_d.ap())
                if s == 1:
                    nc.sync.dma_start(bp2[:], bp2_d.ap())
                nc.sync.dma_start(w2[:, s], w2_d.ap()[:, s])
        else:
            w1 = wpool.tile([128, S, 3, 256], F16)
            for s in range(S):
                x8_tiles[s] = xpool.tile([128, 3, cap], F16, tag="x",
                                         name=f"x{s}")

            def dma_species(s):
                if s < 2:
                    # head-critical species: split each transfer into
                    # 32-partition pieces so the per-partition descriptors
                    # spread across many DMA queues and land ~4x sooner
                    for k in range(3):
                        for p in range(0, 128, 32):
                            nc.sync.dma_start(w1[p : p + 32, s, k],
                                              w1_d.ap()[p : p + 32, s, k])
                            nc.sync.dma_start(
                                x8_tiles[s][p : p + 32, k],
                                xt_d.ap()[p : p + 32, s, k])
                else:
                    nc.sync.dma_start(w1[:, s], w1_d.ap()[:, s])
                    for k in range(3):
                        nc.sync.dma_start(x8_tiles[s][:, k],
                                          xt_d.ap()[:, s, k])
                if s == 0:
                    nc.sync.dma_start(bb[:], bb_d.ap())
                if s == 1:
                    nc.sync.dma_start(bp2[:], bp2_d.ap())
                nc.sync.dma_start(w2[:, s], w2_d.ap()[:, s])

        for s in range(S):
            dma_species(s)
        for s in range(S):
            nc.sync.dma_start(w3[:, s], w3_d.ap()[:, s])
        nc.sync.dma_start(w4[:], w4_d.ap())
        nc.sync.dma_start(bq3[:], bq3_d.ap())

        en_sb = epool.tile([128, len(SGROUPS), cap], F32)

        # --- celu ----------------------------------------------------------
        form_idx = 0

        def tensor_tensor(out, in0, in1, op):
            """Plain DVE tensor-tensor op (2x mode with all-fp16 operands;
            no public bass wrapper)."""
            return nc.vector.add_instruction(
                mybir.InstTensorTensor(
                    name=nc.get_next_instruction_name(),
                    op=op,
                    ins=[nc.vector.lower_ap(in0), nc.vector.lower_ap(in1)],
                    outs=[nc.vector.lower_ap(out)],
                )
            )

        def celu(z, u_out, bx, bc, br):
            """u_out = max(z + bc, min(exp(10 z + bx), alpha)); z PSUM fp32
            view [p, w], u_out SBUF fp16 view [p, w]."""
            nonlocal form_idx
            form = FORM_PATTERN[form_idx % len(FORM_PATTERN)]
            form_idx += 1
            p = z.shape[0]
            w = z.shape[-1]
            e = tpool.tile([128, 1024], F16, tag="e")
            ev = e[:p, :w]
            nc.scalar.activation(ev, z, AF.Exp, bias=bx, scale=10.0)
            if form == "V4":
                # u = min(e, alpha) + relu(z + br); the fused stt form runs
                # at 1x (no 2x uop for STT), so separate min (4x) + tt (2x)
                # is cheaper
                r = tpool.tile([128, 1024], F16, tag="r")
                rv = r[:p, :w]
                nc.scalar.activation(rv, z, AF.Relu, bias=br, scale=1.0)
                mt = tpool.tile([128, 1024], F16, tag="mt")
                mv = mt[:p, :w]
                nc.vector.tensor_scalar(mv, ev, ALPHA, None, op0=ALU.min)
                tensor_tensor(u_out, mv, rv, ALU.add)
            else:
                mt = tpool.tile([128, 1024], F16, tag="mt")
                mv = mt[:p, :w]
                nc.vector.tensor_scalar(mv, ev, ALPHA, None, op0=ALU.min)
                nc.vector.scalar_tensor_tensor(
                    u_out, z, bc, mv, op0=ALU.add, op1=ALU.max
                )

        def celu_s(z, u_out, s, layer, m):
            p = z.shape[0]
            celu(z, u_out,
                 bb[:p, s, layer, 0, m : m + 1],
                 bb[:p, s, layer, 1, m : m + 1],
                 bb[:p, s, layer, 2, m : m + 1])

        # --- layer emitters ------------------------------------------------
        def emit_l1(s):
            u1 = u1pool.tile([128, 2, cap], F16, tag="u1")
            for mi in range(2):
                z = zpool.tile([128, 1024], F32, tag="z")
                msl = slice(mi * 128, mi * 128 + 128)
                for ho, hw in halves:
                    if FP8_L1:
                        # fp16 k2 first (earliest DMA), then DoubleRow (k0,k1)
                        nc.tensor.matmul(
                            z[:, ho : ho + hw],
                            w116[:, s, msl],
                            x16_tiles[s][:, ho : ho + hw],
                            start=True,
                            stop=False,
                        )
                        nc.tensor.matmul(
                            z[:, ho : ho + hw],
                            w18[:, s, :, msl],
                            x8_tiles[s][:, :, ho : ho + hw],
                            start=False,
                            stop=True,
                            perf_mode=PM.DoubleRow,
                        )
                    else:
                        for ki, (ko, kw) in enumerate(l1_k):
                            nc.tensor.matmul(
                                z[:, ho : ho + hw],
                                w1[:, s, ki, msl],
                                x8_tiles[s][:, ki, ho : ho + hw],
                                start=(ki == 0),
                                stop=(ki == 2),
                            )
                celu_s(z[:, :cap], u1[:, mi, :], s, 0, mi)
            return u1

        def emit_l2m0(s, u1):
            u2 = u2pool.tile([128, cap], F16, tag="u2")
            z = zpool.tile([128, 1024], F32, tag="z")
            for ho, hw in halves:
                for ki, (ko, kw) in enumerate(l2_k):
                    nc.tensor.matmul(
                        z[:, ho : ho + hw],
                        w2[:, s, ki, 0:128],
                        u1[:, ki, ho : ho + hw],
                        start=(ki == 0),
                        stop=(ki == 1),
                    )
            celu_s(z[:, :cap], u2[:], s, 1, 0)
            return u2

        def emit_l2m1_packed(pair, zp):
            # consecutive MMs write disjoint 64-col groups of the PE array
            # (tile_position auto-derived from psum base partition) -> they
            # execute concurrently when no waits intervene
            for ho, hw in halves:
                for ki, (ko, kw) in enumerate(l2_k):
                    for slot, s in enumerate(pair):
                        nc.tensor.matmul(
                            zp[64 * slot : 64 * slot + 64, ho : ho + hw],
                            w2[:, s, ki, 128:192],
                            u1s[s][:, ki, ho : ho + hw],
                            start=(ki == 0),
                            stop=(ki == 1),
                        )

        def emit_l2m1_celu(pair, zp):
            pi = pair[0] // 2
            u2p = u2ppool.tile([128, cap], F16, tag="u2p")
            npart = 64 * len(pair)
            celu(zp[:npart, :cap], u2p[:npart, :],
                 bp2[:npart, pi, 0:1], bp2[:npart, pi, 1:2],
                 bp2[:npart, pi, 2:3])
            return u2p

        def emit_l3m0_group(group):
            """L3 m0 for 1-2 species: k0 full-array (serial), k1 row-packed
            (even species rows 0-63, odd rows 64-127 -> concurrent)."""
            zs = {}
            for s in group:
                zs[s] = zpool.tile([128, 1024], F32, tag="z",
                                   name=f"z3m0_{s}")
            for ho, hw in halves:
                for s in group:
                    nc.tensor.matmul(
                        zs[s][:, ho : ho + hw],
                        w3[:, s, 0, 0:128],
                        u2m0s[s][:, ho : ho + hw],
                        start=True,
                        stop=False,
                    )
                for s in group:
                    po = 64 * (s % 2)
                    nc.tensor.matmul(
                        zs[s][:, ho : ho + hw],
                        w3[po : po + 64, s, 1, 0:128],
                        u2pairs[s // 2][po : po + 64, ho : ho + hw],
                        start=False,
                        stop=True,
                    )
            for s in group:
                u3 = u3pool.tile([128, cap], F16, tag="u3", name=f"u3_{s}")
                celu_s(zs[s][:, :cap], u3[:], s, 2, 0)
                u3m0s[s] = u3

        def emit_l3q(quad, u2m0s, u2pairs):
            """Quad-packed L3 m1 (32 rows each) into one z tile; k0s hit
            disjoint 32-col groups and k1s disjoint (row, col) tiles, so
            each batch of 4 runs concurrently in the PE array."""
            qi = quad[0] // 4
            zq = zpool.tile([128, 1024], F32, tag="z", name="zquad")
            for ho, hw in halves:
                for j, s in enumerate(quad):
                    nc.tensor.matmul(
                        zq[32 * j : 32 * j + 32, ho : ho + hw],
                        w3[:, s, 0, 128:160],
                        u2m0s[s][:, ho : ho + hw],
                        start=True,
                        stop=False,
                        tile_position=(0, 32 * j),
                    )
                for j, s in enumerate(quad):
                    po = 64 * (s % 2)
                    nc.tensor.matmul(
                        zq[32 * j : 32 * j + 32, ho : ho + hw],
                        w3[po : po + 64, s, 1, 128:160],
                        u2pairs[s // 2][po : po + 64, ho : ho + hw],
                        start=False,
                        stop=True,
                        tile_position=(po, 32 * j),
                    )
            u3q = u3qpool.tile([128, cap], F16, tag="u3q")
            npart = 32 * len(quad)
            celu(zq[:npart, :cap], u3q[:npart, :],
                 bq3[:npart, qi, 0:1], bq3[:npart, qi, 1:2],
                 bq3[:npart, qi, 2:3])
            return u3q

        # --- emission: pair fronts pipelined with supergroup L3/L4 ----------
        u1s, u2m0s, u2pairs, u3m0s, u3qs = {}, {}, {}, {}, {}

        def emit_front_l1(pair):
            for s in pair:
                u1s[s] = emit_l1(s)

        def emit_front_l2(pair):
            zp = zpool.tile([128, 1024], F32, tag="z", name="zpair")
            for s in pair:
                u2m0s[s] = emit_l2m0(s, u1s[s])
            emit_l2m1_packed(pair, zp)
            u2pairs[pair[0] // 2] = emit_l2m1_celu(pair, zp)

        def emit_front(pair):
            # all L1 matmuls before any L2 so the in-order PE stream never
            # waits on an L1 celu that was emitted moments earlier; the pair
            # m1 matmuls of species A pad the gap before species B's L2
            emit_front_l1(pair)
            emit_front_l2(pair)

        z4s = {}

        def emit_l4_k0(gi, js):
            """k0 matmuls for species-index subset js of supergroup gi
            (disjoint col groups -> concurrent); can run as soon as the
            species' u3m0 is ready, ahead of the rest of the supergroup."""
            quad = QUADS[gi]
            if gi not in z4s:
                z4s[gi] = zpool.tile([128, 1024], F32, tag="z", name="z4")
            z4 = z4s[gi]
            for ho, hw in halves:
                for j in js:
                    s = quad[j]
                    nc.tensor.matmul(
                        z4[32 * j : 32 * j + 32, ho : ho + hw],
                        w4[:, s, 0, :],
                        u3m0s[s][:, ho : ho + hw],
                        start=True,
                        stop=False,
                        tile_position=(0, 32 * j),
                    )

        def emit_l4sg(gi):
            quad = QUADS[gi]
            z4 = z4s[gi]
            u3q = u3qs[gi]
            np_ = 32 * len(quad)
            h = 512
            for ho, hw in halves:
                for j, s in enumerate(quad):  # k1: disjoint (row, col) tiles
                    qo = 32 * j
                    nc.tensor.matmul(
                        z4[32 * j : 32 * j + 32, ho : ho + hw],
                        w4[qo : qo + 32, s, 1, :],
                        u3q[qo : qo + 32, ho : ho + hw],
                        start=False,
                        stop=True,
                        tile_position=(qo, 32 * j),
                    )
                if gi == 1 and ho == 0:
                    # final drain only: copy the finished 512-col half while
                    # the 368-col matmuls still run (for sg0 this would
                    # delay queued celu work on ACT, so keep it after)
                    nc.scalar.copy(en_sb[:np_, gi, :h], z4[:np_, :h])
            # PSUM->SBUF copies (species live at partition blocks 32j),
            # column-split across ACT and DVE
            if gi != 1:
                nc.scalar.copy(en_sb[:np_, gi, :h], z4[:np_, :h])
            nc.vector.tensor_copy(en_sb[:np_, gi, h:cap], z4[:np_, h:cap])
            # ship only the real rows (the 32-row blocks are replicas)
            for j, s in enumerate(quad):
                nc.sync.dma_start(en_d.ap()[s : s + 1],
                                  en_sb[32 * j : 32 * j + 1, gi])

        # Globally interleaved schedule: every celu -> consumer edge gets
        # ~3us of independent PE work so the in-order PE stream rarely
        # waits on the ACT/DVE chain (which is ~2.7us after the z stop).
        for s in (0, 1, 2, 3):
            u1s[s] = emit_l1(s)
        emit_front_l2(PAIRS[0])
        u1s[4] = emit_l1(4)
        emit_front_l2(PAIRS[1])
        u1s[5] = emit_l1(5)
        u1s[6] = emit_l1(6)
        u3qs[0] = emit_l3q(QUADS[0], u2m0s, u2pairs)
        emit_l3m0_group((0, 1))
        emit_front_l2(PAIRS[2])
        emit_l3m0_group((2, 3))
        emit_l4_k0(0, (0, 1))
        emit_front_l2(PAIRS[3])
        emit_l4_k0(0, (2, 3))
        emit_l4sg(0)
        emit_l3m0_group((4, 5))
        u3qs[1] = emit_l3q(QUADS[1], u2m0s, u2pairs)
        emit_l4_k0(1, (0, 1))
        emit_l3m0_group((6,))
        emit_l4_k0(1, (2,))
        emit_l4sg(1)

    nc.compile()
    return nc


def _to_pmajor(wt, k_pad, dtype=np.float16):
    """[S, M, K] weights -> [128, S, k_pad//128, M] partition-major."""
    s, m, k = wt.shape
    arr = np.zeros((s, m, k_pad), np.float32)
    arr[:, :, :k] = wt
    out = arr.transpose(2, 0, 1).reshape(k_pad // 128, 128, s, m).transpose(1, 2, 0, 3)
    return np.ascontiguousarray(out.astype(dtype))


def _prep_weights(W1, b1, W2, b2, W3, b3, W4, b4):
    beta1 = b1
    beta2 = b2 - ALPHA * W2.sum(axis=2)
    beta3 = b3 - ALPHA * W3.sum(axis=2)
    ec = (b4[:, 0] - ALPHA * W4[:, 0, :].sum(axis=1)).astype(np.float64)

    def kinds(beta):
        return (10.0 * beta + LN_ALPHA, beta + ALPHA, beta)

    bb = np.zeros((128, S, 3, 3, 2), np.float32)
    for li, beta in enumerate((beta1, beta2, beta3)):
        m = beta.shape[1]
        pad = np.zeros((S, 256), np.float32)
        pad[:, :m] = beta
        for k, arr in enumerate(kinds(pad)):
            for mi in range(2):
                bb[:, :, li, k, mi] = arr[:, mi * 128 : mi * 128 + 128].T

    bp2 = np.zeros((128, len(PAIRS), 3), np.float32)
    for pi, pair in enumerate(PAIRS):
        for slot, s in enumerate(pair):
            sl = slice(64 * slot, 64 * slot + 64)
            for k, arr in enumerate(kinds(beta2[s][128:192])):
                bp2[sl, pi, k] = arr

    bq3 = np.zeros((128, len(QUADS), 3), np.float32)
    for qi, quad in enumerate(QUADS):
        for j, s in enumerate(quad):
            sl = slice(32 * j, 32 * j + 32)
            for k, arr in enumerate(kinds(beta3[s][128:160])):
                bq3[sl, qi, k] = arr

    # w3/w4 k-tile1 replicated so packed slots can slice at their own base
    # partition (matmul requires lhsT/rhs base partitions to match); w4 also
    # replicated 32x along M so L4 fills full 32-partition output blocks
    w3t = _to_pmajor(W3, 256)
    w3t[64:128, :, 1, :] = w3t[0:64, :, 1, :]
    w4t1 = _to_pmajor(W4, 256)  # [128, S, 2, 1]
    w4t = np.ascontiguousarray(np.broadcast_to(w4t1, (128, S, 2, 32)))
    w4t = w4t.copy()
    for j in range(1, 4):
        w4t[32 * j : 32 * j + 32, :, 1, :] = w4t[0:32, :, 1, :]

    out = dict(
        w2t=_to_pmajor(W2, 256),
        w3t=w3t,
        w4t=w4t,
        biases=bb, bp2=bp2, bq3=bq3,
    )
    if FP8_L1:
        w1t = _to_pmajor(W1, 384)  # [128, S, 3, 256] f16
        out["w1t8"] = np.ascontiguousarray(
            w1t[:, :, 0:2].astype(mybir.dt.np(F8)))
        out["w1t16"] = np.ascontiguousarray(w1t[:, :, 2])
    else:
        out["w1t"] = _to_pmajor(W1, 384)
    return out, ec


def kernel(species, aev, W1, b1, W2, b2, W3, b3, W4, b4):
    global LAST_EXEC_NS
    species = np.asarray(species)
    aev = np.asarray(aev, dtype=np.float32)
    args = [np.asarray(x, dtype=np.float32)
            for x in (W1, b1, W2, b2, W3, b3, W4, b4)]

    sp = species.reshape(-1)
    aev_f = aev.reshape(-1, F)

    # --- balanced atom routing: deal each species round-robin to cores ---
    idx_by_s = [np.nonzero(sp == s)[0] for s in range(S)]
    core_lists = [[idx_by_s[s][c::NCORES] for s in range(S)]
                  for c in range(NCORES)]
    max_n = max(len(core_lists[c][s]) for c in range(NCORES) for s in range(S))
    cap = int(((max_n + 15) // 16) * 16)

    wp, ec = _prep_weights(*args)

    key = (cap, FORM_PATTERN, ZBUFS, TBUFS, U1BUFS, U2BUFS, U2P_BUFS, U3BUFS,
           U3Q_BUFS, WARMUP_MMS, FP8_L1)
    if key not in _CACHE:
        _CACHE[key] = _build(cap)
    nc = _CACHE[key]

    in_maps = []
    for c in range(NCORES):
        xt = np.zeros((128, S, 3, cap), np.float16)
        for s in range(S):
            idx = core_lists[c][s]
            n = len(idx)
            blk = aev_f[idx].T.astype(np.float16)  # [384, n]
            xt[:, s, :, :n] = blk.reshape(3, 128, n).transpose(1, 0, 2)
        if FP8_L1:
            in_maps.append({
                "xt8": np.ascontiguousarray(
                    xt[:, :, 0:2].astype(mybir.dt.np(F8))),
                "xt16": np.ascontiguousarray(xt[:, :, 2]),
                **wp,
            })
        else:
            in_maps.append({"xt": xt, **wp})

    trace = bool(os.environ.get("KERNEL_TRACE"))
    res = run_bass_kernel_spmd(nc, in_maps, list(range(NCORES)), trace=trace)
    LAST_EXEC_NS = res.exec_time_ns

    # --- host reduction ---
    atom_e = np.empty(B * A, np.float64)
    for c in range(NCORES):
        en = np.asarray(res.results[c]["energy"], np.float64)  # [S, cap]
        for s in range(S):
            idx = core_lists[c][s]
            atom_e[idx] = en[s, : len(idx)] + ec[s]
    return atom_e.reshape(B, A).sum(axis=1).astype(np.float32)



# revision 49
# speedup vs baseline: 1.3809x; 1.3809x over previous
"""ANI-style species-routed MLP (MoE routing) on 8 TRN2 NeuronCores, v3.

Strategy (v3, on top of the v2 baseline):
- Atom-level balanced routing: each species' atoms are dealt round-robin
  across the 8 cores, so every core sees ~ceil(N_s/8) atoms of species s;
  capacity is 16-aligned (880 here).  The per-molecule reduction happens
  on the host, so atoms can be assigned to cores freely.
- Feature-major fp16 matmuls (fp32 PSUM accumulate).  An optional fp8
  path (FP8_L1) runs L1's k0/k1 as one fp8e4 DoubleRow matmul (measured
  2x PE rate at N=512) with k2 kept fp16; it is off by default: it saves
  ~1us on average but costs 1.1e-2 relative error and makes the PE
  outrun the PSUM z-tile rotation (celu chain ~2.7us vs 4-deep pool),
  which re-throttles the HAM clock.
- PE warm-up: 8 dummy matmuls with no DMA dependencies run in the input
  DMA shadow, so the HAM activity window (4096 cycles @1.2GHz) completes
  and the PE runs the entire real stream at 2.4GHz (verified: single
  51us warm stretch in the trace).  The exp/relu ACT table set is also
  preloaded there (otherwise it loads lazily, ~1.3us on first celu).
- CELU via the exact exp/min/max trick:
      u := celu(z+b)+a = max(z + b + a, min(a*e^{10(z+b)}, a))
                       = min(a*e^{10(z+b)}, a) + relu(z+b)
  with V1 (exp ACT; min 4x + stt 1x on DVE) and V4 (exp+relu ACT; min +
  tt-add on DVE) cycled by FORM_PATTERN to balance ACT ~= DVE ~= 44us.
  (scalar_tensor_tensor always runs at 1x -- fusing min+add into one stt
  is a loss; GpSimd tensor ops are ~10-40x slower than DVE on TRN2.)
- Small-M/K matmuls exploit PE 32x32 sub-array concurrency: consecutive
  matmuls on disjoint (row,col) tile_position groups execute with ~3ns
  stagger (measured).  L2 m1 (M=64) packs 2 species on col groups
  {0,64}; L3/L4 m1 (M=32) pack 4 species on col groups {0,32,64,96};
  L3/L4 k1 (K=64/32) also row-tile.  Matmul order within a group is
  k0(all species) then k1(all species) so the concurrent tiles are
  adjacent in the instruction stream with no waits between them.
- Globally interleaved emission: every celu -> consumer edge gets ~3us
  of independent PE work (the celu chain exp->min->stt is ~2.7us after
  the z stop), including splitting L4's k0 matmuls out so they run as
  soon as each species' u3 is ready.
- Energy output ships one [1, cap] row per species (the 32-row blocks
  are replicas); supergroup-0 rows are DMA'd mid-kernel.
"""
import os
import sys

sys.path.insert(0, "/opt/trn_rl_repo")

from contextlib import ExitStack

import numpy as np

import concourse.bass as bass
import concourse.mybir as mybir
import concourse.tile as tile
from concourse import bacc
from concourse.bass_utils import run_bass_kernel_spmd

F32 = mybir.dt.float32
F16 = mybir.dt.float16
F8 = mybir.dt.float8e4
PM = mybir.MatmulPerfMode
AF = mybir.ActivationFunctionType
ALU = mybir.AluOpType

B, A, F = 1024, 48, 384
S = 7
NCORES = 8
ALPHA = 0.1
LN_ALPHA = float(np.log(ALPHA))

PAIRS = ((0, 1), (2, 3), (4, 5), (6,))
QUADS = ((0, 1, 2, 3), (4, 5, 6))
SGROUPS = ((0, 1), (2, 3))  # supergroups: pair indices per L3/L4 block

# --- tuning knobs (cache key includes them) ---
FORM_PATTERN = ("V1", "V4", "V1", "V1", "V1", "V4", "V1",
                "V1", "V4", "V1", "V1", "V1", "V4", "V1")
ZBUFS = 4
TBUFS = 8
U1BUFS = 4
U2BUFS = 7
U2P_BUFS = 4
U3BUFS = 7
U3Q_BUFS = 2
WARMUP_MMS = 8
FP8_L1 = False  # L1 weights+aev in fp8e4 with DoubleRow (k0,k1) + normal k2

_CACHE = {}
LAST_EXEC_NS = None


def _build(cap):
    assert cap % 16 == 0
    halves = [(o, min(512, cap - o)) for o in range(0, cap, 512)]
    nc = bacc.Bacc()

    if FP8_L1:
        # k0,k1 in fp8 (DoubleRow pair); k2 stays fp16 for accuracy --
        # same PE cost, ~sqrt(2/3) of the full-fp8 quantization error
        x8_d = nc.declare_dram_parameter("xt8", [128, S, 2, cap], F8,
                                         isOutput=False)
        w18_d = nc.declare_dram_parameter("w1t8", [128, S, 2, 256], F8,
                                          isOutput=False)
        x16_d = nc.declare_dram_parameter("xt16", [128, S, cap], F16,
                                          isOutput=False)
        w116_d = nc.declare_dram_parameter("w1t16", [128, S, 256], F16,
                                           isOutput=False)
    else:
        xt_d = nc.declare_dram_parameter("xt", [128, S, 3, cap], F16,
                                         isOutput=False)
        w1_d = nc.declare_dram_parameter("w1t", [128, S, 3, 256], F16,
                                         isOutput=False)
    w2_d = nc.declare_dram_parameter("w2t", [128, S, 2, 192], F16, isOutput=False)
    w3_d = nc.declare_dram_parameter("w3t", [128, S, 2, 160], F16, isOutput=False)
    # w4 columns replicated 32x: L4 outputs fill a full 32-partition block
    # (M=32 costs the same as M=1) so the PSUM->SBUF copy reads no stale rows
    w4_d = nc.declare_dram_parameter("w4t", [128, S, 2, 32], F16, isOutput=False)
    # biases [128, S, layer(3), kind(bx,bc,br), m(2)] (m1 slots of L2/L3 unused)
    bb_d = nc.declare_dram_parameter("biases", [128, S, 3, 3, 2], F32,
                                     isOutput=False)
    # packed L2-m1 pair biases [128, npairs, kind]
    bp2_d = nc.declare_dram_parameter("bp2", [128, len(PAIRS), 3], F32,
                                      isOutput=False)
    # packed L3-m1 quad biases [128, nquads, kind]
    bq3_d = nc.declare_dram_parameter("bq3", [128, len(QUADS), 3], F32,
                                      isOutput=False)
    # energy out: one row per species (partition 32*(s%4) of supergroup s//4)
    en_d = nc.declare_dram_parameter("energy", [S, cap], F32, isOutput=True)

    l1_k = [(0, 128), (128, 128), (256, 128)]
    l2_k = [(0, 128), (128, 128)]
    l3_k = [(0, 128), (128, 64)]
    l4_k = [(0, 128), (128, 32)]

    with tile.TileContext(nc) as tc, ExitStack() as ctx:
        wpool = ctx.enter_context(tc.tile_pool(name="weights", bufs=1))
        xpool = ctx.enter_context(tc.tile_pool(name="x", bufs=S))
        u1pool = ctx.enter_context(tc.tile_pool(name="u1", bufs=U1BUFS))
        u2pool = ctx.enter_context(tc.tile_pool(name="u2", bufs=U2BUFS))
        u2ppool = ctx.enter_context(tc.tile_pool(name="u2p", bufs=U2P_BUFS))
        u3pool = ctx.enter_context(tc.tile_pool(name="u3", bufs=U3BUFS))
        u3qpool = ctx.enter_context(tc.tile_pool(name="u3q", bufs=U3Q_BUFS))
        tpool = ctx.enter_context(tc.tile_pool(name="t", bufs=TBUFS))
        zpool = ctx.enter_context(tc.tile_pool(name="z", bufs=ZBUFS, space="PSUM"))
        epool = ctx.enter_context(tc.tile_pool(name="en", bufs=1))

        # --- PE warm-up: dummy matmuls with no DMA dependency keep the HAM
        # activity window busy from kernel start, so the ramp to K=8/8
        # happens during the input-DMA shadow instead of eating into the
        # real matmul stream. ---
        wdummy = wpool.tile([128, 512], F16)
        nc.vector.memset(wdummy[:], 0.0)
        zwarm = zpool.tile([128, 1024], F32, tag="z", name="zwarm")
        for _ in range(WARMUP_MMS):
            nc.tensor.matmul(zwarm[:, 0:512], wdummy[:, 0:128],
                             wdummy[:], start=True, stop=True)
        # preload the exp/relu ACT table set during the DMA shadow (it
        # otherwise loads lazily at the first celu, ~1.3us on the critical
        # path)
        ed = wpool.tile([128, 8], F16)
        nc.scalar.activation(ed[:], zwarm[:, 0:8], AF.Exp, bias=0.0,
                             scale=1.0)

        # --- DMAs: ordered by first-use time.  Species 0's x/w1 chunks are
        # k-interleaved and issued first so the first L1 matmul can start as
        # early as possible; w3/w4/bq3 are only needed by the tails and go
        # last so they don't starve the early matmul stream. ---
        bb = wpool.tile([128, S, 3, 3, 2], F32)
        bp2 = wpool.tile([128, len(PAIRS), 3], F32)
        bq3 = wpool.tile([128, len(QUADS), 3], F32)

        w2 = wpool.tile([128, S, 2, 192], F16)
        w3 = wpool.tile([128, S, 2, 160], F16)
        w4 = wpool.tile([128, S, 2, 32], F16)
        x8_tiles, x16_tiles = {}, {}
        if FP8_L1:
            w18 = wpool.tile([128, S, 2, 256], F8)
            w116 = wpool.tile([128, S, 256], F16)
            for s in range(S):
                x8_tiles[s] = xpool.tile([128, 2, cap], F8, tag="x8",
                                         name=f"x8_{s}")
                x16_tiles[s] = xpool.tile([128, cap], F16, tag="x16",
                                          name=f"x16_{s}")

            def dma_species(s):
                # fp16 k2 first: the first matmul of each species reads it
                nc.sync.dma_start(w116[:, s], w116_d.ap()[:, s])
                nc.sync.dma_start(x16_tiles[s][:], x16_d.ap()[:, s])
                nc.sync.dma_start(w18[:, s], w18_d.ap()[:, s])
                nc.sync.dma_start(x8_tiles[s][:], x8_d.ap()[:, s])
                if s == 0:
                    nc.sync.dma_start(bb[:], bb_d.ap())
                if s == 1:
                    nc.sync.dma_start(bp2[:], bp2_d.ap())
                nc.sync.dma_start(w2[:, s], w2_d.ap()[:, s])
        else:
            w1 = wpool.tile([128, S, 3, 256], F16)
            for s in range(S):
                x8_tiles[s] = xpool.tile([128, 3, cap], F16, tag="x",
                                         name=f"x{s}")

            def dma_species(s):
                nc.sync.dma_start(w1[:, s], w1_d.ap()[:, s])
                for k in range(3):
                    nc.sync.dma_start(x8_tiles[s][:, k],
                                      xt_d.ap()[:, s, k])
                if s == 0:
                    nc.sync.dma_start(bb[:], bb_d.ap())
                if s == 1:
                    nc.sync.dma_start(bp2[:], bp2_d.ap())
                nc.sync.dma_start(w2[:, s], w2_d.ap()[:, s])

        for s in range(S):
            dma_species(s)
        for s in range(S):
            nc.sync.dma_start(w3[:, s], w3_d.ap()[:, s])
        nc.sync.dma_start(w4[:], w4_d.ap())
        nc.sync.dma_start(bq3[:], bq3_d.ap())

        en_sb = epool.tile([128, len(SGROUPS), cap], F32)

        # --- celu ----------------------------------------------------------
        form_idx = 0

        def tensor_tensor(out, in0, in1, op):
            """Plain DVE tensor-tensor op (2x mode with all-fp16 operands;
            no public bass wrapper)."""
            return nc.vector.add_instruction(
                mybir.InstTensorTensor(
                    name=nc.get_next_instruction_name(),
                    op=op,
                    ins=[nc.vector.lower_ap(in0), nc.vector.lower_ap(in1)],
                    outs=[nc.vector.lower_ap(out)],
                )
            )

        def celu(z, u_out, bx, bc, br):
            """u_out = max(z + bc, min(exp(10 z + bx), alpha)); z PSUM fp32
            view [p, w], u_out SBUF fp16 view [p, w]."""
            nonlocal form_idx
            form = FORM_PATTERN[form_idx % len(FORM_PATTERN)]
            form_idx += 1
            p = z.shape[0]
            w = z.shape[-1]
            e = tpool.tile([128, 1024], F16, tag="e")
            ev = e[:p, :w]
            nc.scalar.activation(ev, z, AF.Exp, bias=bx, scale=10.0)
            if form == "V4":
                # u = min(e, alpha) + relu(z + br); the fused stt form runs
                # at 1x (no 2x uop for STT), so separate min (4x) + tt (2x)
                # is cheaper
                r = tpool.tile([128, 1024], F16, tag="r")
                rv = r[:p, :w]
                nc.scalar.activation(rv, z, AF.Relu, bias=br, scale=1.0)
                mt = tpool.tile([128, 1024], F16, tag="mt")
                mv = mt[:p, :w]
                nc.vector.tensor_scalar(mv, ev, ALPHA, None, op0=ALU.min)
                tensor_tensor(u_out, mv, rv, ALU.add)
            else:
                mt = tpool.tile([128, 1024], F16, tag="mt")
                mv = mt[:p, :w]
                nc.vector.tensor_scalar(mv, ev, ALPHA, None, op0=ALU.min)
                nc.vector.scalar_tensor_tensor(
                    u_out, z, bc, mv, op0=ALU.add, op1=ALU.max
                )

        def celu_s(z, u_out, s, layer, m):
            p = z.shape[0]
            celu(z, u_out,
                 bb[:p, s, layer, 0, m : m + 1],
                 bb[:p, s, layer, 1, m : m + 1],
                 bb[:p, s, layer, 2, m : m + 1])

        # --- layer emitters ------------------------------------------------
        def emit_l1(s):
            u1 = u1pool.tile([128, 2, cap], F16, tag="u1")
            for mi in range(2):
                z = zpool.tile([128, 1024], F32, tag="z")
                msl = slice(mi * 128, mi * 128 + 128)
                for ho, hw in halves:
                    if FP8_L1:
                        # fp16 k2 first (earliest DMA), then DoubleRow (k0,k1)
                        nc.tensor.matmul(
                            z[:, ho : ho + hw],
                            w116[:, s, msl],
                            x16_tiles[s][:, ho : ho + hw],
                            start=True,
                            stop=False,
                        )
                        nc.tensor.matmul(
                            z[:, ho : ho + hw],
                            w18[:, s, :, msl],
                            x8_tiles[s][:, :, ho : ho + hw],
                            start=False,
                            stop=True,
                            perf_mode=PM.DoubleRow,
                        )
                    else:
                        for ki, (ko, kw) in enumerate(l1_k):
                            nc.tensor.matmul(
                                z[:, ho : ho + hw],
                                w1[:, s, ki, msl],
                                x8_tiles[s][:, ki, ho : ho + hw],
                                start=(ki == 0),
                                stop=(ki == 2),
                            )
                celu_s(z[:, :cap], u1[:, mi, :], s, 0, mi)
            return u1

        def emit_l2m0(s, u1):
            u2 = u2pool.tile([128, cap], F16, tag="u2")
            z = zpool.tile([128, 1024], F32, tag="z")
            for ho, hw in halves:
                for ki, (ko, kw) in enumerate(l2_k):
                    nc.tensor.matmul(
                        z[:, ho : ho + hw],
                        w2[:, s, ki, 0:128],
                        u1[:, ki, ho : ho + hw],
                        start=(ki == 0),
                        stop=(ki == 1),
                    )
            celu_s(z[:, :cap], u2[:], s, 1, 0)
            return u2

        def emit_l2m1_packed(pair, zp):
            # consecutive MMs write disjoint 64-col groups of the PE array
            # (tile_position auto-derived from psum base partition) -> they
            # execute concurrently when no waits intervene
            for ho, hw in halves:
                for ki, (ko, kw) in enumerate(l2_k):
                    for slot, s in enumerate(pair):
                        nc.tensor.matmul(
                            zp[64 * slot : 64 * slot + 64, ho : ho + hw],
                            w2[:, s, ki, 128:192],
                            u1s[s][:, ki, ho : ho + hw],
                            start=(ki == 0),
                            stop=(ki == 1),
                        )

        def emit_l2m1_celu(pair, zp):
            pi = pair[0] // 2
            u2p = u2ppool.tile([128, cap], F16, tag="u2p")
            npart = 64 * len(pair)
            celu(zp[:npart, :cap], u2p[:npart, :],
                 bp2[:npart, pi, 0:1], bp2[:npart, pi, 1:2],
                 bp2[:npart, pi, 2:3])
            return u2p

        def emit_l3m0_group(group):
            """L3 m0 for 1-2 species: k0 full-array (serial), k1 row-packed
            (even species rows 0-63, odd rows 64-127 -> concurrent)."""
            zs = {}
            for s in group:
                zs[s] = zpool.tile([128, 1024], F32, tag="z",
                                   name=f"z3m0_{s}")
            for ho, hw in halves:
                for s in group:
                    nc.tensor.matmul(
                        zs[s][:, ho : ho + hw],
                        w3[:, s, 0, 0:128],
                        u2m0s[s][:, ho : ho + hw],
                        start=True,
                        stop=False,
                    )
                for s in group:
                    po = 64 * (s % 2)
                    nc.tensor.matmul(
                        zs[s][:, ho : ho + hw],
                        w3[po : po + 64, s, 1, 0:128],
                        u2pairs[s // 2][po : po + 64, ho : ho + hw],
                        start=False,
                        stop=True,
                    )
            for s in group:
                u3 = u3pool.tile([128, cap], F16, tag="u3", name=f"u3_{s}")
                celu_s(zs[s][:, :cap], u3[:], s, 2, 0)
                u3m0s[s] = u3

        def emit_l3q(quad, u2m0s, u2pairs):
            """Quad-packed L3 m1 (32 rows each) into one z tile; k0s hit
            disjoint 32-col groups and k1s disjoint (row, col) tiles, so
            each batch of 4 runs concurrently in the PE array."""
            qi = quad[0] // 4
            zq = zpool.tile([128, 1024], F32, tag="z", name="zquad")
            for ho, hw in halves:
                for j, s in enumerate(quad):
                    nc.tensor.matmul(
                        zq[32 * j : 32 * j + 32, ho : ho + hw],
                        w3[:, s, 0, 128:160],
                        u2m0s[s][:, ho : ho + hw],
                        start=True,
                        stop=False,
                        tile_position=(0, 32 * j),
                    )
                for j, s in enumerate(quad):
                    po = 64 * (s % 2)
                    nc.tensor.matmul(
                        zq[32 * j : 32 * j + 32, ho : ho + hw],
                        w3[po : po + 64, s, 1, 128:160],
                        u2pairs[s // 2][po : po + 64, ho : ho + hw],
                        start=False,
                        stop=True,
                        tile_position=(po, 32 * j),
                    )
            u3q = u3qpool.tile([128, cap], F16, tag="u3q")
            npart = 32 * len(quad)
            celu(zq[:npart, :cap], u3q[:npart, :],
                 bq3[:npart, qi, 0:1], bq3[:npart, qi, 1:2],
                 bq3[:npart, qi, 2:3])
            return u3q

        # --- emission: pair fronts pipelined with supergroup L3/L4 ----------
        u1s, u2m0s, u2pairs, u3m0s, u3qs = {}, {}, {}, {}, {}

        def emit_front_l1(pair):
            for s in pair:
                u1s[s] = emit_l1(s)

        def emit_front_l2(pair):
            zp = zpool.tile([128, 1024], F32, tag="z", name="zpair")
            for s in pair:
                u2m0s[s] = emit_l2m0(s, u1s[s])
            emit_l2m1_packed(pair, zp)
            u2pairs[pair[0] // 2] = emit_l2m1_celu(pair, zp)

        def emit_front(pair):
            # all L1 matmuls before any L2 so the in-order PE stream never
            # waits on an L1 celu that was emitted moments earlier; the pair
            # m1 matmuls of species A pad the gap before species B's L2
            emit_front_l1(pair)
            emit_front_l2(pair)

        z4s = {}

        def emit_l4_k0(gi, js):
            """k0 matmuls for species-index subset js of supergroup gi
            (disjoint col groups -> concurrent); can run as soon as the
            species' u3m0 is ready, ahead of the rest of the supergroup."""
            quad = QUADS[gi]
            if gi not in z4s:
                z4s[gi] = zpool.tile([128, 1024], F32, tag="z", name="z4")
            z4 = z4s[gi]
            for ho, hw in halves:
                for j in js:
                    s = quad[j]
                    nc.tensor.matmul(
                        z4[32 * j : 32 * j + 32, ho : ho + hw],
                        w4[:, s, 0, :],
                        u3m0s[s][:, ho : ho + hw],
                        start=True,
                        stop=False,
                        tile_position=(0, 32 * j),
                    )

        def emit_l4sg(gi):
            quad = QUADS[gi]
            z4 = z4s[gi]
            u3q = u3qs[gi]
            np_ = 32 * len(quad)
            h = 512
            for ho, hw in halves:
                for j, s in enumerate(quad):  # k1: disjoint (row, col) tiles
                    qo = 32 * j
                    nc.tensor.matmul(
                        z4[32 * j : 32 * j + 32, ho : ho + hw],
                        w4[qo : qo + 32, s, 1, :],
                        u3q[qo : qo + 32, ho : ho + hw],
                        start=False,
                        stop=True,
                        tile_position=(qo, 32 * j),
                    )
                if gi == 1 and ho == 0:
                    # final drain only: copy the finished 512-col half while
                    # the 368-col matmuls still run (for sg0 this would
                    # delay queued celu work on ACT, so keep it after)
                    nc.scalar.copy(en_sb[:np_, gi, :h], z4[:np_, :h])
            # PSUM->SBUF copies (species live at partition blocks 32j),
            # column-split across ACT and DVE
            if gi != 1:
                nc.scalar.copy(en_sb[:np_, gi, :h], z4[:np_, :h])
            nc.vector.tensor_copy(en_sb[:np_, gi, h:cap], z4[:np_, h:cap])
            # ship only the real rows (the 32-row blocks are replicas)
            for j, s in enumerate(quad):
                nc.sync.dma_start(en_d.ap()[s : s + 1],
                                  en_sb[32 * j : 32 * j + 1, gi])

        # Globally interleaved schedule: every celu -> consumer edge gets
        # ~3us of independent PE work so the in-order PE stream rarely
        # waits on the ACT/DVE chain (which is ~2.7us after the z stop).
        for s in (0, 1, 2, 3):
            u1s[s] = emit_l1(s)
        emit_front_l2(PAIRS[0])
        u1s[4] = emit_l1(4)
        emit_front_l2(PAIRS[1])
        u1s[5] = emit_l1(5)
        u1s[6] = emit_l1(6)
        u3qs[0] = emit_l3q(QUADS[0], u2m0s, u2pairs)
        emit_l3m0_group((0, 1))
        emit_front_l2(PAIRS[2])
        emit_l3m0_group((2, 3))
        emit_l4_k0(0, (0, 1))
        emit_front_l2(PAIRS[3])
        emit_l4_k0(0, (2, 3))
        emit_l4sg(0)
        emit_l3m0_group((4, 5))
        u3qs[1] = emit_l3q(QUADS[1], u2m0s, u2pairs)
        emit_l4_k0(1, (0, 1))
        emit_l3m0_group((6,))
        emit_l4_k0(1, (2,))
        emit_l4sg(1)

    nc.compile()
    return nc


def _to_pmajor(wt, k_pad, dtype=np.float16):
    """[S, M, K] weights -> [128, S, k_pad//128, M] partition-major."""
    s, m, k = wt.shape
    arr = np.zeros((s, m, k_pad), np.float32)
    arr[:, :, :k] = wt
    out = arr.transpose(2, 0, 1).reshape(k_pad // 128, 128, s, m).transpose(1, 2, 0, 3)
    return np.ascontiguousarray(out.astype(dtype))


def _prep_weights(W1, b1, W2, b2, W3, b3, W4, b4):
    beta1 = b1
    beta2 = b2 - ALPHA * W2.sum(axis=2)
    beta3 = b3 - ALPHA * W3.sum(axis=2)
    ec = (b4[:, 0] - ALPHA * W4[:, 0, :].sum(axis=1)).astype(np.float64)

    def kinds(beta):
        return (10.0 * beta + LN_ALPHA, beta + ALPHA, beta)

    bb = np.zeros((128, S, 3, 3, 2), np.float32)
    for li, beta in enumerate((beta1, beta2, beta3)):
        m = beta.shape[1]
        pad = np.zeros((S, 256), np.float32)
        pad[:, :m] = beta
        for k, arr in enumerate(kinds(pad)):
            for mi in range(2):
                bb[:, :, li, k, mi] = arr[:, mi * 128 : mi * 128 + 128].T

    bp2 = np.zeros((128, len(PAIRS), 3), np.float32)
    for pi, pair in enumerate(PAIRS):
        for slot, s in enumerate(pair):
            sl = slice(64 * slot, 64 * slot + 64)
            for k, arr in enumerate(kinds(beta2[s][128:192])):
                bp2[sl, pi, k] = arr

    bq3 = np.zeros((128, len(QUADS), 3), np.float32)
    for qi, quad in enumerate(QUADS):
        for j, s in enumerate(quad):
            sl = slice(32 * j, 32 * j + 32)
            for k, arr in enumerate(kinds(beta3[s][128:160])):
                bq3[sl, qi, k] = arr

    # w3/w4 k-tile1 replicated so packed slots can slice at their own base
    # partition (matmul requires lhsT/rhs base partitions to match); w4 also
    # replicated 32x along M so L4 fills full 32-partition output blocks
    w3t = _to_pmajor(W3, 256)
    w3t[64:128, :, 1, :] = w3t[0:64, :, 1, :]
    w4t1 = _to_pmajor(W4, 256)  # [128, S, 2, 1]
    w4t = np.ascontiguousarray(np.broadcast_to(w4t1, (128, S, 2, 32)))
    w4t = w4t.copy()
    for j in range(1, 4):
        w4t[32 * j : 32 * j + 32, :, 1, :] = w4t[0:32, :, 1, :]

    out = dict(
        w2t=_to_pmajor(W2, 256),
        w3t=w3t,
        w4t=w4t,
        biases=bb, bp2=bp2, bq3=bq3,
    )
    if FP8_L1:
        w1t = _to_pmajor(W1, 384)  # [128, S, 3, 256] f16
        out["w1t8"] = np.ascontiguousarray(
            w1t[:, :, 0:2].astype(mybir.dt.np(F8)))
        out["w1t16"] = np.ascontiguousarray(w1t[:, :, 2])
    else:
        out["w1t"] = _to_pmajor(W1, 384)
    return out, ec


def kernel(species, aev, W1, b1, W2, b2, W3, b3, W4, b4):
    global LAST_EXEC_NS
    species = np.asarray(species)
    aev = np.asarray(aev, dtype=np.float32)
    args = [np.asarray(x, dtype=np.float32)
            for x in (W1, b1, W2, b2, W3, b3, W4, b4)]

    sp = species.reshape(-1)
    aev_f = aev.reshape(-1, F)

    # --- balanced atom routing: deal each species round-robin to cores ---
    idx_by_s = [np.nonzero(sp == s)[0] for s in range(S)]
    core_lists = [[idx_by_s[s][c::NCORES] for s in range(S)]
                  for c in range(NCORES)]
    max_n = max(len(core_lists[c][s]) for c in range(NCORES) for s in range(S))
    cap = int(((max_n + 15) // 16) * 16)

    wp, ec = _prep_weights(*args)

    key = (cap, FORM_PATTERN, ZBUFS, TBUFS, U1BUFS, U2BUFS, U2P_BUFS, U3BUFS,
           U3Q_BUFS, WARMUP_MMS, FP8_L1)
    if key not in _CACHE:
        _CACHE[key] = _build(cap)
    nc = _CACHE[key]

    in_maps = []
    for c in range(NCORES):
        xt = np.zeros((128, S, 3, cap), np.float16)
        for s in range(S):
            idx = core_lists[c][s]
            n = len(idx)
            blk = aev_f[idx].T.astype(np.float16)  # [384, n]
            xt[:, s, :, :n] = blk.reshape(3, 128, n).transpose(1, 0, 2)
        if FP8_L1:
            in_maps.append({
                "xt8": np.ascontiguousarray(
                    xt[:, :, 0:2].astype(mybir.dt.np(F8))),
                "xt16": np.ascontiguousarray(xt[:, :, 2]),
                **wp,
            })
        else:
            in_maps.append({"xt": xt, **wp})

    trace = bool(os.environ.get("KERNEL_TRACE"))
    res = run_bass_kernel_spmd(nc, in_maps, list(range(NCORES)), trace=trace)
    LAST_EXEC_NS = res.exec_time_ns

    # --- host reduction ---
    atom_e = np.empty(B * A, np.float64)
    for c in range(NCORES):
        en = np.asarray(res.results[c]["energy"], np.float64)  # [S, cap]
        for s in range(S):
            idx = core_lists[c][s]
            atom_e[idx] = en[s, : len(idx)] + ec[s]
    return atom_e.reshape(B, A).sum(axis=1).astype(np.float32)

